# revision 44
# baseline (speedup 1.0000x reference)
"""Multi-head attention (B=2, H=8, T=4096, C=64, fp32) on 8 Trainium2 cores.

Sharding: batch*heads = 16 head-blocks, 2 per core (head-parallel, no
cross-core communication). Per head-block each core computes
    out = softmax(Q K^T / sqrt(C)) V
with a transposed-scores dataflow (scores^T[s, t] tiles in PSUM).

v4 structure (vs the 369us TC=512 pair-packed version):
  - weight loads are SHARED: a standalone nc.tensor.ldweights() feeds
    two 512-col matmuls whose InstMatmult.ldweights=False, halving the
    PE's stationary-load traffic (128 rows per 1024 score cols, 65 rows
    per 1024 PV cols).  Matmul outputs stay 512 f32 per partition (one
    PSUM bank) to satisfy the s3d3 ISA check.
  - V' ([V | ones]) and pt (exp scores) are bf16: standalone ldweights
    is only legal for <=2-byte dtypes, PV becomes an all-bf16 matmul,
    and pt SBUF halves.
  - PSUM: psc tag ring 2 x [128,2,512] f32 (4 banks; prologue transpose
    batches ride the same ring) + psm tags accL/accR 2 x 1 bank each,
    with the epilogue td4 tiles sharing those rings (4 banks) = 8.
  - K^T/Q^T layouts are [64, T] bf16.  Head-0 K + chunk-0 Q transpose
    on the PE (batched through psc + one batched copy); everything else
    (head-0 Q chunks 1-3, all of head-1) uses the DMA XBAR transpose on
    the sync queue — no PE, PSUM, or Scalar/Vector involvement — with
    one partition-shift SBUF DMA per parity to assemble [64, T].
  - softmax exp split across Scalar (ACTIVATE Exp table) and Vector
    (Schraudolph tensor_scalar + custom-DVE mantissa polish) per
    [128,1024] tile.
"""

from contextlib import ExitStack

import numpy as np

B, H, T_FULL, C = 2, 8, 4096, 64
N_CORES = 8
HPC = (B * H) // N_CORES  # head-blocks per core

# --- fast-exp constants (fit offline; see transcript). The correction is
# normalized to a*(z-z0)^2 + 1 with the former g0 factor folded into the
# Schraudolph bias.
_LOG2E = 1.4426950408889634
_B_C = 1064624065.4483186
_CC = 0.2481254275845736
_Z0 = 1.48530923
_MASK = float(np.int32(0x007FFFFF).view(np.float32))

# stream assignment per 32 tiles: counts of (scalar-exact, dve-corrected)
N_S, N_C = 22, 10


def _make_assignment(n_slot, ns, ncr):
    """Interleave ns 'S' and ncr 'C' over n_slot slots, round-robin by
    largest remainder so each stream's work is spread through the chunk."""
    tot = ns + ncr
    out = []
    acc = {"S": 0.0, "C": 0.0}
    w = {"S": ns / tot, "C": ncr / tot}
    for _ in range(n_slot):
        for k in acc:
            acc[k] += w[k]
        pick = max(acc, key=lambda k: acc[k])
        acc[pick] -= 1.0
        out.append(pick)
    return out


def _register_exp_op():
    import concourse.dve_ops as dvo
    from concourse.dve_spec import (
        AluOp,
        Bin,
        C0,
        C1,
        C2,
        One,
        Spec,
        Src0,
        lower,
        sq,
    )
    from concourse.dve_uop import DveOpSpec

    name = "EXP_CORRECT_ANT"
    if name in dvo._SUB_OPCODE_FOR_NAME:
        return next(op for op in dvo.OPS if op.name == name)

    def _ref(in0, in1, s0, s1, imm2):
        i = np.asarray(in0, np.float32).view(np.int32)
        z = ((i & 0x007FFFFF) | 0x3F800000).view(np.float32)
        d = z - np.float32(s1)
        g = (d * d * np.float32(imm2) + np.float32(1.0)).astype(np.float32)
        return (np.asarray(in0, np.float32) * g).astype(np.float32)

    body = Src0 * (
        sq(Bin(AluOp.BITWISE_OR, Bin(AluOp.BITWISE_AND, Src0, C0), One) - C1) * C2
        + One
    )
    spec = Spec(body=body, reference=_ref)
    row = dvo._CUSTOM_DVE_ROW_BASE + len(dvo.OPS)
    assert row < 0x20
    dvo._SUB_OPCODE_FOR_NAME[name] = row
    shas = {}
    for ver in ("v3",):
        uops = lower(spec, ver=ver)
        shas[ver] = DveOpSpec(name=name, opcode=row, uops=uops, rd1_en=False).sha(
            ver
        )
    op = dvo.DveOp(name, spec, subdim=False, uops_sha=shas)
    dvo.OPS.append(op)
    dvo.CUSTOM_DVE_SPECS[name] = spec
    return op


def build_attention_bass(T=T_FULL, heads=HPC, split=(N_S, N_C)):
    import concourse.bass as bass
    import concourse.tile as tile
    from concourse import bacc, mybir
    from concourse.masks import make_identity

    exp_op = _register_exp_op()

    f32 = mybir.dt.float32
    bf16 = mybir.dt.bfloat16
    i32 = mybir.dt.int32
    P = 128
    HALF = 512                  # columns per matmul (one PSUM bank of f32)
    TCB = min(1024, T)          # t-chunk (columns per shared weight load)
    NH = TCB // HALF            # matmuls per weight load
    SB = 128                    # s-block (rows per score matmul output)
    n_tc = T // TCB
    n_sb = T // SB
    n_tt = T // P               # t-tiles per head
    LAG = 4

    # exp stream constants (0.125 score scale folded in)
    SC_A = 0.125 * _LOG2E * 8388608.0
    BC_C = _B_C + 0.5  # +0.5 makes truncate-on-convert equal round-to-nearest

    assign = _make_assignment(n_sb, *split)

    nc = bacc.Bacc(
        "TRN2", target_bir_lowering=False, debug=False, num_devices=N_CORES
    )

    q_d = nc.dram_tensor("q", [heads, T, C], f32, kind="ExternalInput").ap()
    k_d = nc.dram_tensor("k", [heads, T, C], f32, kind="ExternalInput").ap()
    v_d = nc.dram_tensor("v", [heads, T, C], f32, kind="ExternalInput").ap()
    o_d = nc.dram_tensor("out", [heads, T, C], f32, kind="ExternalOutput").ap()

    with tile.TileContext(nc) as tc, ExitStack() as ctx:
        const_pool = ctx.enter_context(tc.tile_pool(name="const", bufs=1))
        stage_pool = ctx.enter_context(tc.tile_pool(name="stage", bufs=4))
        qkt_pool = ctx.enter_context(tc.tile_pool(name="qkt", bufs=4))
        vp_pool = ctx.enter_context(tc.tile_pool(name="vp", bufs=2))
        pt_pool = ctx.enter_context(tc.tile_pool(name="pt", bufs=6))
        it_pool = ctx.enter_context(tc.tile_pool(name="it", bufs=3))
        accT_pool = ctx.enter_context(tc.tile_pool(name="accT", bufs=3))
        out_pool = ctx.enter_context(tc.tile_pool(name="outsb", bufs=3))
        rec_pool = ctx.enter_context(tc.tile_pool(name="rec", bufs=3))
        psc = ctx.enter_context(tc.tile_pool(name="psc", bufs=2, space="PSUM"))
        psm = ctx.enter_context(tc.tile_pool(name="psm", bufs=2, space="PSUM"))

        # identities: bulk memset on the (idle) Vector engine; only the
        # cheap affine_select rides gpsimd, slotted between staging kicks.
        identb = const_pool.tile([P, P], bf16, tag="identb")
        ident = const_pool.tile([P, P], f32, tag="ident")
        nc.vector.memset(identb[:], 0.0)
        nc.vector.memset(ident[:], 0.0)

        def ident_select(t):
            nc.gpsimd.affine_select(
                out=t,
                in_=t,
                compare_op=mybir.AluOpType.not_equal,
                fill=1.0,
                base=0,
                pattern=[[-1, P]],
                channel_multiplier=1,
            )

        nq = n_tt // 4  # staging quarter
        q_sts, k_sts, vsbs = [], [], []
        for h in range(heads):
            q_st = stage_pool.tile([P, n_tt, C], bf16, tag="stage")
            k_st = stage_pool.tile([P, n_tt, C], bf16, tag="stage")
            q_sts.append(q_st); k_sts.append(k_st)
            # V' = [V | ones] per s-block, bf16 so PV weight loads can be
            # shared (standalone ldweights rejects 4-byte dtypes); only
            # the ones-column needs the memset (DMA fills the rest)
            v_sb = vp_pool.tile([P, n_sb, C + 1], bf16, tag="vp")
            vsbs.append(v_sb)
            nc.vector.memset(v_sb[:, :, C : C + 1], 1.0)

        def kick_qk(st, src, lo, hi):
            nc.gpsimd.dma_start(
                st[:, lo:hi, :],
                src.rearrange("(n p) c -> p n c", p=P)[:, lo:hi, :],
            )

        def kick_v(h):
            v_rr = v_d[h].rearrange("(n p) c -> p n c", p=P)
            nc.gpsimd.dma_start(vsbs[h][:, :, 0:C], v_rr[:])

        # gpsimd staging order, by first use (software-DGE transfers run
        # serially in kick order): K-q0 and Q-q0 first so chunk 0's first
        # score groups start ~13us; remaining K quarters land while their
        # transposes ride chunk-0 slots 0/2/4; V-0 halves before the
        # first PV (~3 slots in); head 1 last.
        kick_qk(k_sts[0], k_d[0], 0, nq)
        ident_select(identb[:])
        kick_qk(q_sts[0], q_d[0], 0, nq)
        kick_qk(k_sts[0], k_d[0], nq, 2 * nq)
        kick_qk(k_sts[0], k_d[0], 2 * nq, 3 * nq)
        kick_qk(k_sts[0], k_d[0], 3 * nq, 4 * nq)
        kick_v(0)
        kick_qk(q_sts[0], q_d[0], nq, 4 * nq)
        ident_select(ident[:])
        for h in range(1, heads):
            kick_qk(k_sts[h], k_d[h], 0, n_tt // 2)
            kick_qk(k_sts[h], k_d[h], n_tt // 2, n_tt)
            kick_v(h)
            kick_qk(q_sts[h], q_d[h], 0, n_tt // 2)
            kick_qk(q_sts[h], q_d[h], n_tt // 2, n_tt)

        kts, qts = [], []
        for h in range(heads):
            kt = qkt_pool.tile([C, T], bf16, tag="qkt")
            qt = qkt_pool.tile([C, T], bf16, tag="qkt")
            kts.append(kt); qts.append(qt)

        def pe_transposes(h, which, j0, n):
            """PE-transpose staged tiles j0..j0+n-1 into qt/kt [64, T]:
            n transposes into one psc-ring tile + one batched copy."""
            st = (q_sts if which == "q" else k_sts)[h]
            dst = (qts if which == "q" else kts)[h]
            tp = psc.tile([C, n, P], bf16, tag="sc")
            for jj in range(n):
                nc.tensor.transpose(tp[:, jj, :], st[:, j0 + jj, :], identb[:])
            if (j0 // n) % 2 == 0:
                nc.scalar.activation(
                    dst[:, j0 * P : (j0 + n) * P],
                    tp[:],
                    mybir.ActivationFunctionType.Copy,
                )
            else:
                nc.vector.tensor_copy(dst[:, j0 * P : (j0 + n) * P], tp[:])

        NB = min(8, n_tt)  # transpose batch size
        # head 0: K^T quarter 0 + Q^T chunk 0 — the minimum to start
        # chunk 0 — via PE transposes (NB-tile batches, psc ring).
        pe_transposes(0, "k", 0, NB)
        for j0 in range(0, TCB // P, NB):
            pe_transposes(0, "q", j0, NB)

        # Everything else rides the same PE path, NB-tile batches emitted
        # at fixed slots inside head-0's main loop (the psc-ring
        # insertion steals one lookahead step — a short stall each).
        # K quarters 1-3 go in chunk-0's first slots, chasing staging.
        late_work = []
        for j0 in range(NB, n_sb, NB):
            late_work.append((0, "k", j0))
        for j0 in range(TCB // P, n_tt, NB):
            late_work.append((0, "q", j0))
        for h in range(1, heads):
            for j0 in range(0, n_tt, NB):
                late_work.append((h, "k", j0))
        for h in range(1, heads):
            for j0 in range(0, n_tt, NB):
                late_work.append((h, "q", j0))
        late_slots = (12, 20, 28) if n_sb >= 32 else (1, 2, 3)

        def emit_late(item):
            pe_transposes(*item, NB)

        # ---- main loops (heads sequential; LAG s-blocks of score
        # lookahead hide the exp latency before the PV matmuls consume
        # each pt).  Each chunk's epilogue (per 512-col half) is DEFERRED
        # into the next chunk's slot stream: per half X in L,R: stage 0
        # copy acc->accT, stage 1 transposes + reciprocal, stage 2
        # normalize + out DMA.
        def emit_epilogue(h, i, q, acc, stage, state, final=False):
            if stage == 0:
                accT = accT_pool.tile([C + 1, HALF], f32, tag="accT")
                nc.scalar.activation(
                    accT[:], acc[:], mybir.ActivationFunctionType.Copy
                )
                state["accT"] = accT
            elif stage == 1:
                accT = state["accT"]
                td4 = psm.tile(
                    [P, HALF // P, C + 1], f32, tag="accL" if q == 0 else "accR"
                )
                for b in range(HALF // P):
                    nc.tensor.transpose(
                        td4[:, b, :],
                        accT[:, b * P : (b + 1) * P],
                        ident[0 : C + 1, 0 : C + 1],
                    )
                rec = rec_pool.tile([P, HALF // P, 1], f32, tag="rec")
                nc.vector.reciprocal(rec[:], td4[:, :, C : C + 1])
                state["td4"] = td4
                state["rec"] = rec
            else:
                td4, rec = state["td4"], state["rec"]
                osb = out_pool.tile([P, HALF // P, C], f32, tag="outsb")
                for b in range(HALF // P):
                    if final and b % 2 == 0:
                        # exposed at the kernel tail: split the normalize
                        # across Scalar and Vector
                        nc.scalar.activation(
                            osb[:, b, :],
                            td4[:, b, 0:C],
                            mybir.ActivationFunctionType.Copy,
                            scale=rec[:, b, :],
                        )
                    else:
                        nc.vector.tensor_scalar_mul(
                            osb[:, b, :],
                            td4[:, b, 0:C],
                            rec[:, b, :],
                        )
                o_r = o_d[h].rearrange("(n p) c -> p n c", p=P)
                nb = HALF // P
                nc.sync.dma_start(
                    o_r[:, (i * NH + q) * nb : (i * NH + q + 1) * nb, :], osb[:]
                )

        for h in range(heads):
            kt, qt, v_sb = kts[h], qts[h], vsbs[h]

            def emit_score(i, m, pts):
                sc = psc.tile([P, NH, HALF], f32, tag="sc")
                w = kt[:, m * SB : (m + 1) * SB]
                nc.tensor.ldweights(w)
                for q in range(NH):
                    mi = nc.tensor.matmul(
                        sc[:, q, :],
                        lhsT=w,
                        rhs=qt[:, i * TCB + q * HALF : i * TCB + (q + 1) * HALF],
                        start=True,
                        stop=True,
                    )
                    mi.ins.ldweights = False
                pt = pt_pool.tile([P, NH, HALF], bf16, tag="pt")
                if assign[m] == "S":
                    nc.scalar.activation(
                        pt[:],
                        sc[:],
                        mybir.ActivationFunctionType.Exp,
                        scale=0.125,
                    )
                else:  # corrected: affine+convert, then mantissa polish
                    it = it_pool.tile([P, NH, HALF], i32, tag="it")
                    nc.vector.tensor_scalar(
                        it[:],
                        sc[:],
                        SC_A,
                        BC_C,
                        op0=mybir.AluOpType.mult,
                        op1=mybir.AluOpType.add,
                    )
                    nc.vector._custom_dve(
                        exp_op,
                        out=pt[:],
                        in0=it[:].bitcast(f32),
                        s0=_MASK,
                        s1=_Z0,
                        imm2=_CC,
                    )
                pts[(i, m)] = pt

            pending = None  # (i, [accs]) of the prev chunk awaiting epilogue
            pts = {}
            LAGe = min(LAG, n_sb)
            for m in range(LAGe):  # warm-up: first LAGe blocks of chunk 0
                emit_score(0, m, pts)
            for i in range(n_tc):
                accs = [
                    psm.tile(
                        [C + 1, HALF], f32,
                        tag="accL" if q == 0 else "accR",
                        name=f"acc{q}",
                    )
                    for q in range(NH)
                ]
                epi_states = [dict() for _ in range(NH)]
                for m in range(n_sb):
                    # emit the score block LAGe ahead (maybe next chunk's)
                    t = m + LAGe
                    if t < n_sb:
                        emit_score(i, t, pts)
                    elif i + 1 < n_tc:
                        emit_score(i + 1, t - n_sb, pts)
                    if pending is not None and m < 3 * NH:
                        emit_epilogue(
                            h, pending[0], m // 3, pending[1][m // 3],
                            m % 3, epi_states[m // 3],
                        )
                        if m == 3 * NH - 1:
                            pending = None
                    if h == 0 and late_work and (
                        m in late_slots or (i == 0 and m in (0, 2, 4))
                    ):
                        emit_late(late_work.pop(0))
                    pt = pts.pop((i, m))
                    wv = v_sb[:, m, :]
                    nc.tensor.ldweights(wv)
                    for q in range(NH):
                        mi = nc.tensor.matmul(
                            accs[q][:],
                            lhsT=wv,
                            rhs=pt[:, q, :],
                            start=(m == 0),
                            stop=(m == n_sb - 1),
                        )
                        mi.ins.ldweights = False
                pending = (i, accs)

            # flush the head's last chunk
            final = h == heads - 1
            epi_states = [dict() for _ in range(NH)]
            for q in range(NH):
                for st in range(3):
                    emit_epilogue(
                        h, pending[0], q, pending[1][q], st, epi_states[q],
                        final=final and q == NH - 1,
                    )

    nc.compile()
    return nc


_NC_CACHE = {}


def _get_nc(T, heads):
    key = (T, heads, N_S, N_C)
    if key not in _NC_CACHE:
        _NC_CACHE[key] = build_attention_bass(T, heads)
    return _NC_CACHE[key]


def _install_ntff_hook():
    """Register the axon NTFF profile hook that this image's antenv lacks.
    Only used when kernel(trace=True); never on the grading path."""
    import sys
    import types

    try:
        from antenv.axon_hooks import get_axon_ntff_profile_hook  # noqa: F401

        return
    except ImportError:
        pass
    import antenv
    from trn_agent_boot.trn_boot import _ntff_profile_via_ctypes

    holder = [_ntff_profile_via_ctypes("/opt/axon/libaxon_pjrt.so")]
    mod = types.ModuleType("antenv.axon_hooks")
    mod.get_axon_ntff_profile_hook = lambda: holder[0]
    mod.set_axon_ntff_profile_hook = lambda h: holder.__setitem__(0, h)
    sys.modules["antenv.axon_hooks"] = mod
    antenv.axon_hooks = mod

    import concourse.bass_utils as bu

    bu.upload_artifacts = lambda tmpdir: tmpdir  # no bucket in this sandbox


def kernel(query, key, value, trace=False):
    from concourse.bass_utils import run_bass_kernel_spmd

    if trace:
        _install_ntff_hook()

    Bq, Hq, T, Cq = query.shape
    nh = Bq * Hq
    heads = nh // N_CORES
    q = np.ascontiguousarray(query.reshape(nh, T, Cq).astype(np.float32))
    k = np.ascontiguousarray(key.reshape(nh, T, Cq).astype(np.float32))
    v = np.ascontiguousarray(value.reshape(nh, T, Cq).astype(np.float32))

    nc = _get_nc(T, heads)
    in_maps = [
        {
            "q": q[i * heads : (i + 1) * heads],
            "k": k[i * heads : (i + 1) * heads],
            "v": v[i * heads : (i + 1) * heads],
        }
        for i in range(N_CORES)
    ]
    res = run_bass_kernel_spmd(
        nc, in_maps, core_ids=list(range(N_CORES)), trace=trace
    )
    out = np.concatenate([res.results[i]["out"] for i in range(N_CORES)], axis=0)
    if trace:
        kernel.last_results = res
    return out.reshape(Bq, Hq, T, Cq)


# revision 48
# speedup vs baseline: 1.1386x; 1.1386x over previous
"""Multi-head attention (B=2, H=8, T=4096, C=64, fp32) on 8 Trainium2 cores.

Sharding: batch*heads = 16 head-blocks, 2 per core (head-parallel, no
cross-core communication). Per head-block each core computes
    out = softmax(Q K^T / sqrt(C)) V
with a transposed-scores dataflow (scores^T[s, t] tiles in PSUM).

Structure (measured ~348-354us on HW vs the 369-382us TC=512 baseline):
  - weight loads are SHARED: a standalone nc.tensor.ldweights() feeds
    two 512-col matmuls whose InstMatmult.ldweights=False, halving the
    PE's stationary-load traffic (128 rows per 1024 score cols, 65 rows
    per 1024 PV cols).  Matmul outputs stay 512 f32 per partition (one
    PSUM bank) to satisfy the s3d3 ISA check.
  - V' ([V | ones]) and pt (exp scores) are bf16: standalone ldweights
    is only legal for <=2-byte dtypes, PV becomes an all-bf16 matmul,
    and pt SBUF halves.  Measured rel err vs the fp32 reference:
    1.129e-2 (deterministic inputs; harness gate is 2e-2).
  - PSUM: psc tag ring 2 x [128,2,512] f32 (4 banks; prologue transpose
    batches ride the same ring) + psm tags accL/accR 2 x 1 bank each,
    with the epilogue td4 tiles sharing those rings (4 banks) = 8.
  - K^T/Q^T layouts are [64, T] bf16, built by PE transposes in 8-tile
    batches through the psc ring + one batched casting copy each.
    Head-0's K quarter 0 + Q chunk 0 run upfront (first score ~13us);
    all remaining batches are emitted at fixed slots inside head-0's
    main loop, chasing the gpsimd staging DMAs.
  - softmax exp split 22:10 across Scalar (ACTIVATE Exp table) and
    Vector (Schraudolph tensor_scalar + custom-DVE mantissa polish)
    per [128,1024] score tile; identity masks are memset on Vector
    with only the affine_select on gpsimd.
"""

from contextlib import ExitStack

import numpy as np

B, H, T_FULL, C = 2, 8, 4096, 64
N_CORES = 8
HPC = (B * H) // N_CORES  # head-blocks per core

# --- fast-exp constants (fit offline; see transcript). The correction is
# normalized to a*(z-z0)^2 + 1 with the former g0 factor folded into the
# Schraudolph bias.
_LOG2E = 1.4426950408889634
_B_C = 1064624065.4483186
_CC = 0.2481254275845736
_Z0 = 1.48530923
_MASK = float(np.int32(0x007FFFFF).view(np.float32))

# stream assignment per 32 tiles: counts of (scalar-exact, dve-corrected)
N_S, N_C = 22, 10


def _make_assignment(n_slot, ns, ncr):
    """Interleave ns 'S' and ncr 'C' over n_slot slots, round-robin by
    largest remainder so each stream's work is spread through the chunk."""
    tot = ns + ncr
    out = []
    acc = {"S": 0.0, "C": 0.0}
    w = {"S": ns / tot, "C": ncr / tot}
    for _ in range(n_slot):
        for k in acc:
            acc[k] += w[k]
        pick = max(acc, key=lambda k: acc[k])
        acc[pick] -= 1.0
        out.append(pick)
    return out


def _register_exp_op():
    import concourse.dve_ops as dvo
    from concourse.dve_spec import (
        AluOp,
        Bin,
        C0,
        C1,
        C2,
        One,
        Spec,
        Src0,
        lower,
        sq,
    )
    from concourse.dve_uop import DveOpSpec

    name = "EXP_CORRECT_ANT"
    if name in dvo._SUB_OPCODE_FOR_NAME:
        return next(op for op in dvo.OPS if op.name == name)

    def _ref(in0, in1, s0, s1, imm2):
        i = np.asarray(in0, np.float32).view(np.int32)
        z = ((i & 0x007FFFFF) | 0x3F800000).view(np.float32)
        d = z - np.float32(s1)
        g = (d * d * np.float32(imm2) + np.float32(1.0)).astype(np.float32)
        return (np.asarray(in0, np.float32) * g).astype(np.float32)

    body = Src0 * (
        sq(Bin(AluOp.BITWISE_OR, Bin(AluOp.BITWISE_AND, Src0, C0), One) - C1) * C2
        + One
    )
    spec = Spec(body=body, reference=_ref)
    row = dvo._CUSTOM_DVE_ROW_BASE + len(dvo.OPS)
    assert row < 0x20
    dvo._SUB_OPCODE_FOR_NAME[name] = row
    shas = {}
    for ver in ("v3",):
        uops = lower(spec, ver=ver)
        shas[ver] = DveOpSpec(name=name, opcode=row, uops=uops, rd1_en=False).sha(
            ver
        )
    op = dvo.DveOp(name, spec, subdim=False, uops_sha=shas)
    dvo.OPS.append(op)
    dvo.CUSTOM_DVE_SPECS[name] = spec
    return op


def build_attention_bass(T=T_FULL, heads=HPC, split=(N_S, N_C)):
    import concourse.bass as bass
    import concourse.tile as tile
    from concourse import bacc, mybir
    from concourse.masks import make_identity

    exp_op = _register_exp_op()

    f32 = mybir.dt.float32
    bf16 = mybir.dt.bfloat16
    i32 = mybir.dt.int32
    P = 128
    HALF = 512                  # columns per matmul (one PSUM bank of f32)
    TCB = min(1024, T)          # t-chunk (columns per shared weight load)
    NH = TCB // HALF            # matmuls per weight load
    SB = 128                    # s-block (rows per score matmul output)
    n_tc = T // TCB
    n_sb = T // SB
    n_tt = T // P               # t-tiles per head
    LAG = 3

    # exp stream constants (0.125 score scale folded in)
    SC_A = 0.125 * _LOG2E * 8388608.0
    BC_C = _B_C + 0.5  # +0.5 makes truncate-on-convert equal round-to-nearest

    assign = _make_assignment(n_sb, *split)

    nc = bacc.Bacc(
        "TRN2", target_bir_lowering=False, debug=False, num_devices=N_CORES
    )

    q_d = nc.dram_tensor("q", [heads, T, C], f32, kind="ExternalInput").ap()
    k_d = nc.dram_tensor("k", [heads, T, C], f32, kind="ExternalInput").ap()
    v_d = nc.dram_tensor("v", [heads, T, C], f32, kind="ExternalInput").ap()
    o_d = nc.dram_tensor("out", [heads, T, C], f32, kind="ExternalOutput").ap()

    with tile.TileContext(nc) as tc, ExitStack() as ctx:
        const_pool = ctx.enter_context(tc.tile_pool(name="const", bufs=1))
        stage_pool = ctx.enter_context(tc.tile_pool(name="stage", bufs=4))
        qkt_pool = ctx.enter_context(tc.tile_pool(name="qkt", bufs=4))
        vp_pool = ctx.enter_context(tc.tile_pool(name="vp", bufs=2))
        pt_pool = ctx.enter_context(tc.tile_pool(name="pt", bufs=5))
        it_pool = ctx.enter_context(tc.tile_pool(name="it", bufs=3))
        accT_pool = ctx.enter_context(tc.tile_pool(name="accT", bufs=3))
        out_pool = ctx.enter_context(tc.tile_pool(name="outsb", bufs=3))
        rec_pool = ctx.enter_context(tc.tile_pool(name="rec", bufs=3))
        psc = ctx.enter_context(tc.tile_pool(name="psc", bufs=2, space="PSUM"))
        psm = ctx.enter_context(tc.tile_pool(name="psm", bufs=2, space="PSUM"))

        # identities: bulk memset on the (idle) Vector engine; only the
        # cheap affine_select rides gpsimd, slotted between staging kicks.
        identb = const_pool.tile([P, P], bf16, tag="identb")
        ident = const_pool.tile([P, P], f32, tag="ident")
        nc.vector.memset(identb[:], 0.0)
        nc.vector.memset(ident[:], 0.0)

        def ident_select(t):
            nc.gpsimd.affine_select(
                out=t,
                in_=t,
                compare_op=mybir.AluOpType.not_equal,
                fill=1.0,
                base=0,
                pattern=[[-1, P]],
                channel_multiplier=1,
            )

        nq = n_tt // 4  # staging quarter
        q_sts, k_sts, vsbs = [], [], []
        for h in range(heads):
            q_st = stage_pool.tile([P, n_tt, C], bf16, tag="stage")
            k_st = stage_pool.tile([P, n_tt, C], bf16, tag="stage")
            q_sts.append(q_st); k_sts.append(k_st)
            # V' = [V | ones] per s-block, bf16 so PV weight loads can be
            # shared (standalone ldweights rejects 4-byte dtypes); only
            # the ones-column needs the memset (DMA fills the rest)
            v_sb = vp_pool.tile([P, n_sb, C + 1], bf16, tag="vp")
            vsbs.append(v_sb)
            nc.vector.memset(v_sb[:, :, C : C + 1], 1.0)

        def kick_qk(st, src, lo, hi):
            nc.gpsimd.dma_start(
                st[:, lo:hi, :],
                src.rearrange("(n p) c -> p n c", p=P)[:, lo:hi, :],
            )

        def kick_v(h):
            v_rr = v_d[h].rearrange("(n p) c -> p n c", p=P)
            nc.gpsimd.dma_start(vsbs[h][:, :, 0:C], v_rr[:])

        # gpsimd staging order, by first use (software-DGE transfers run
        # serially in kick order): K-q0 and Q-q0 first so chunk 0's first
        # score groups start ~13us; remaining K quarters land while their
        # transposes ride chunk-0 slots 0/2/4; V-0 halves before the
        # first PV (~3 slots in); head 1 last.
        kick_qk(k_sts[0], k_d[0], 0, nq)
        ident_select(identb[:])
        kick_qk(q_sts[0], q_d[0], 0, nq)
        kick_qk(k_sts[0], k_d[0], nq, 2 * nq)
        kick_qk(k_sts[0], k_d[0], 2 * nq, 3 * nq)
        kick_qk(k_sts[0], k_d[0], 3 * nq, 4 * nq)
        kick_v(0)
        kick_qk(q_sts[0], q_d[0], nq, 4 * nq)
        ident_select(ident[:])
        for h in range(1, heads):
            kick_qk(k_sts[h], k_d[h], 0, n_tt // 2)
            kick_qk(k_sts[h], k_d[h], n_tt // 2, n_tt)
            kick_v(h)
            kick_qk(q_sts[h], q_d[h], 0, n_tt // 2)
            kick_qk(q_sts[h], q_d[h], n_tt // 2, n_tt)

        kts, qts = [], []
        for h in range(heads):
            kt = qkt_pool.tile([C, T], bf16, tag="qkt")
            qt = qkt_pool.tile([C, T], bf16, tag="qkt")
            kts.append(kt); qts.append(qt)

        def pe_transposes(h, which, j0, n):
            """PE-transpose staged tiles j0..j0+n-1 into qt/kt [64, T]:
            n transposes into one psc-ring tile + one batched copy."""
            st = (q_sts if which == "q" else k_sts)[h]
            dst = (qts if which == "q" else kts)[h]
            tp = psc.tile([C, n, P], bf16, tag="sc")
            for jj in range(n):
                nc.tensor.transpose(tp[:, jj, :], st[:, j0 + jj, :], identb[:])
            if (j0 // n) % 2 == 0:
                nc.scalar.activation(
                    dst[:, j0 * P : (j0 + n) * P],
                    tp[:],
                    mybir.ActivationFunctionType.Copy,
                )
            else:
                nc.vector.tensor_copy(dst[:, j0 * P : (j0 + n) * P], tp[:])

        NB = min(8, n_tt)  # transpose batch size
        # head 0: K^T quarter 0 + Q^T chunk 0 — the minimum to start
        # chunk 0 — via PE transposes (NB-tile batches, psc ring).
        pe_transposes(0, "k", 0, NB)
        for j0 in range(0, TCB // P, NB):
            pe_transposes(0, "q", j0, NB)

        # Everything else rides the same PE path, NB-tile batches emitted
        # at fixed slots inside head-0's main loop (the psc-ring
        # insertion steals one lookahead step — a short stall each).
        # K quarters 1-3 go in chunk-0's first slots, chasing staging.
        late_work = []
        for j0 in range(NB, n_sb, NB):
            late_work.append((0, "k", j0))
        for j0 in range(TCB // P, n_tt, NB):
            late_work.append((0, "q", j0))
        for h in range(1, heads):
            for j0 in range(0, n_tt, NB):
                late_work.append((h, "k", j0))
        for h in range(1, heads):
            for j0 in range(0, n_tt, NB):
                late_work.append((h, "q", j0))
        late_slots = (12, 20, 28) if n_sb >= 32 else (1, 2, 3)

        def emit_late(item):
            pe_transposes(*item, NB)

        # ---- main loops (heads sequential; LAG s-blocks of score
        # lookahead hide the exp latency before the PV matmuls consume
        # each pt).  Each chunk's epilogue (per 512-col half) is DEFERRED
        # into the next chunk's slot stream: per half X in L,R: stage 0
        # copy acc->accT, stage 1 transposes + reciprocal, stage 2
        # normalize + out DMA.
        def emit_epilogue(h, i, q, acc, stage, state, final=False):
            if stage == 0:
                # bf16 epilogue intermediate: halves the PE transpose
                # cost (1 cyc/row vs 2); costs ~0.3% extra rounding on
                # the output (gate is 2e-2)
                accT = accT_pool.tile([C + 1, HALF], bf16, tag="accT")
                nc.scalar.activation(
                    accT[:], acc[:], mybir.ActivationFunctionType.Copy
                )
                state["accT"] = accT
            elif stage == 1:
                accT = state["accT"]
                # inner dim padded to C+2 so each bf16 block starts
                # 4-byte aligned in PSUM (132B stride)
                td4 = psm.tile(
                    [P, HALF // P, C + 2], bf16,
                    tag="accL" if q == 0 else "accR",
                )
                for b in range(HALF // P):
                    nc.tensor.transpose(
                        td4[:, b, 0 : C + 1],
                        accT[:, b * P : (b + 1) * P],
                        identb[0 : C + 1, 0 : C + 1],
                    )
                rec = rec_pool.tile([P, HALF // P, 1], f32, tag="rec")
                nc.vector.reciprocal(rec[:], td4[:, :, C : C + 1])
                state["td4"] = td4
                state["rec"] = rec
            else:
                td4, rec = state["td4"], state["rec"]
                osb = out_pool.tile([P, HALF // P, C], f32, tag="outsb")
                for b in range(HALF // P):
                    if final and b % 2 == 0:
                        # exposed at the kernel tail: split the normalize
                        # across Scalar and Vector
                        nc.scalar.activation(
                            osb[:, b, :],
                            td4[:, b, 0:C],
                            mybir.ActivationFunctionType.Copy,
                            scale=rec[:, b, :],
                        )
                    else:
                        nc.vector.tensor_scalar_mul(
                            osb[:, b, :],
                            td4[:, b, 0:C],
                            rec[:, b, :],
                        )
                o_r = o_d[h].rearrange("(n p) c -> p n c", p=P)
                nb = HALF // P
                nc.sync.dma_start(
                    o_r[:, (i * NH + q) * nb : (i * NH + q + 1) * nb, :], osb[:]
                )

        for h in range(heads):
            kt, qt, v_sb = kts[h], qts[h], vsbs[h]

            def emit_score(i, m, pts):
                sc = psc.tile([P, NH, HALF], f32, tag="sc")
                w = kt[:, m * SB : (m + 1) * SB]
                nc.tensor.ldweights(w)
                for q in range(NH):
                    mi = nc.tensor.matmul(
                        sc[:, q, :],
                        lhsT=w,
                        rhs=qt[:, i * TCB + q * HALF : i * TCB + (q + 1) * HALF],
                        start=True,
                        stop=True,
                    )
                    mi.ins.ldweights = False
                pt = pt_pool.tile([P, NH, HALF], bf16, tag="pt")
                if assign[m] == "S":
                    nc.scalar.activation(
                        pt[:],
                        sc[:],
                        mybir.ActivationFunctionType.Exp,
                        scale=0.125,
                    )
                else:  # corrected: affine+convert, then mantissa polish
                    it = it_pool.tile([P, NH, HALF], i32, tag="it")
                    nc.vector.tensor_scalar(
                        it[:],
                        sc[:],
                        SC_A,
                        BC_C,
                        op0=mybir.AluOpType.mult,
                        op1=mybir.AluOpType.add,
                    )
                    nc.vector._custom_dve(
                        exp_op,
                        out=pt[:],
                        in0=it[:].bitcast(f32),
                        s0=_MASK,
                        s1=_Z0,
                        imm2=_CC,
                    )
                pts[(i, m)] = pt

            pending = None  # (i, [accs]) of the prev chunk awaiting epilogue
            pts = {}
            LAGe = min(LAG, n_sb)
            for m in range(LAGe):  # warm-up: first LAGe blocks of chunk 0
                emit_score(0, m, pts)
            for i in range(n_tc):
                accs = [
                    psm.tile(
                        [C + 1, HALF], f32,
                        tag="accL" if q == 0 else "accR",
                        name=f"acc{q}",
                    )
                    for q in range(NH)
                ]
                epi_states = [dict() for _ in range(NH)]
                for m in range(n_sb):
                    # emit the score block LAGe ahead (maybe next chunk's)
                    t = m + LAGe
                    if t < n_sb:
                        emit_score(i, t, pts)
                    elif i + 1 < n_tc:
                        emit_score(i + 1, t - n_sb, pts)
                    if pending is not None and m < 3 * NH:
                        emit_epilogue(
                            h, pending[0], m // 3, pending[1][m // 3],
                            m % 3, epi_states[m // 3],
                        )
                        if m == 3 * NH - 1:
                            pending = None
                    if h == 0 and late_work and (
                        m in late_slots or (i == 0 and m in (0, 2, 4))
                    ):
                        emit_late(late_work.pop(0))
                    pt = pts.pop((i, m))
                    wv = v_sb[:, m, :]
                    nc.tensor.ldweights(wv)
                    for q in range(NH):
                        mi = nc.tensor.matmul(
                            accs[q][:],
                            lhsT=wv,
                            rhs=pt[:, q, :],
                            start=(m == 0),
                            stop=(m == n_sb - 1),
                        )
                        mi.ins.ldweights = False
                pending = (i, accs)

            # flush the head's last chunk
            final = h == heads - 1
            epi_states = [dict() for _ in range(NH)]
            for q in range(NH):
                for st in range(3):
                    emit_epilogue(
                        h, pending[0], q, pending[1][q], st, epi_states[q],
                        final=final and q == NH - 1,
                    )

    nc.compile()
    return nc


_NC_CACHE = {}


def _get_nc(T, heads):
    key = (T, heads, N_S, N_C)
    if key not in _NC_CACHE:
        _NC_CACHE[key] = build_attention_bass(T, heads)
    return _NC_CACHE[key]


def _install_ntff_hook():
    """Register the axon NTFF profile hook that this image's antenv lacks.
    Only used when kernel(trace=True); never on the grading path."""
    import sys
    import types

    try:
        from antenv.axon_hooks import get_axon_ntff_profile_hook  # noqa: F401

        return
    except ImportError:
        pass
    import antenv
    from trn_agent_boot.trn_boot import _ntff_profile_via_ctypes

    holder = [_ntff_profile_via_ctypes("/opt/axon/libaxon_pjrt.so")]
    mod = types.ModuleType("antenv.axon_hooks")
    mod.get_axon_ntff_profile_hook = lambda: holder[0]
    mod.set_axon_ntff_profile_hook = lambda h: holder.__setitem__(0, h)
    sys.modules["antenv.axon_hooks"] = mod
    antenv.axon_hooks = mod

    import concourse.bass_utils as bu

    bu.upload_artifacts = lambda tmpdir: tmpdir  # no bucket in this sandbox


def kernel(query, key, value, trace=False):
    from concourse.bass_utils import run_bass_kernel_spmd

    if trace:
        _install_ntff_hook()

    Bq, Hq, T, Cq = query.shape
    nh = Bq * Hq
    heads = nh // N_CORES
    q = np.ascontiguousarray(query.reshape(nh, T, Cq).astype(np.float32))
    k = np.ascontiguousarray(key.reshape(nh, T, Cq).astype(np.float32))
    v = np.ascontiguousarray(value.reshape(nh, T, Cq).astype(np.float32))

    nc = _get_nc(T, heads)
    in_maps = [
        {
            "q": q[i * heads : (i + 1) * heads],
            "k": k[i * heads : (i + 1) * heads],
            "v": v[i * heads : (i + 1) * heads],
        }
        for i in range(N_CORES)
    ]
    res = run_bass_kernel_spmd(
        nc, in_maps, core_ids=list(range(N_CORES)), trace=trace
    )
    out = np.concatenate([res.results[i]["out"] for i in range(N_CORES)], axis=0)
    if trace:
        kernel.last_results = res
    return out.reshape(Bq, Hq, T, Cq)


# revision 51
# speedup vs baseline: 1.3872x; 1.2184x over previous
"""Multi-head attention (B=2, H=8, T=4096, C=64, fp32) on 8 Trainium2 cores.

Sharding: batch*heads = 16 head-blocks, 2 per core (head-parallel, no
cross-core communication). Per head-block each core computes
    out = softmax(Q K^T / sqrt(C)) V
with a transposed-scores dataflow (scores^T[s, t] tiles in PSUM).

Structure (measured ~348-354us on HW vs the 369-382us TC=512 baseline):
  - weight loads are SHARED: a standalone nc.tensor.ldweights() feeds
    two 512-col matmuls whose InstMatmult.ldweights=False, halving the
    PE's stationary-load traffic (128 rows per 1024 score cols, 65 rows
    per 1024 PV cols).  Matmul outputs stay 512 f32 per partition (one
    PSUM bank) to satisfy the s3d3 ISA check.
  - V' ([V | ones]) and pt (exp scores) are bf16: standalone ldweights
    is only legal for <=2-byte dtypes, PV becomes an all-bf16 matmul,
    and pt SBUF halves.  Measured rel err vs the fp32 reference:
    1.129e-2 (deterministic inputs; harness gate is 2e-2).
  - PSUM: psc tag ring 2 x [128,2,512] f32 (4 banks; prologue transpose
    batches ride the same ring) + psm tags accL/accR 2 x 1 bank each,
    with the epilogue td4 tiles sharing those rings (4 banks) = 8.
  - K^T/Q^T layouts are [64, T] bf16, built by PE transposes in 8-tile
    batches through the psc ring + one batched casting copy each.
    Head-0's K quarter 0 + Q chunk 0 run upfront (first score ~13us);
    all remaining batches are emitted at fixed slots inside head-0's
    main loop, chasing the gpsimd staging DMAs.
  - softmax exp split 22:10 across Scalar (ACTIVATE Exp table) and
    Vector (Schraudolph tensor_scalar + custom-DVE mantissa polish)
    per [128,1024] score tile; identity masks are memset on Vector
    with only the affine_select on gpsimd.
"""

from contextlib import ExitStack

import numpy as np

B, H, T_FULL, C = 2, 8, 4096, 64
N_CORES = 8
HPC = (B * H) // N_CORES  # head-blocks per core

# --- fast-exp constants (fit offline; see transcript). The correction is
# normalized to a*(z-z0)^2 + 1 with the former g0 factor folded into the
# Schraudolph bias.
_LOG2E = 1.4426950408889634
_B_C = 1064624065.4483186
_CC = 0.2481254275845736
_Z0 = 1.48530923
_MASK = float(np.int32(0x007FFFFF).view(np.float32))

# stream assignment per 32 tiles: counts of (scalar-exact, dve-corrected)
N_S, N_C = 22, 10


def _make_assignment(n_slot, ns, ncr):
    """Interleave ns 'S' and ncr 'C' over n_slot slots, round-robin by
    largest remainder so each stream's work is spread through the chunk."""
    tot = ns + ncr
    out = []
    acc = {"S": 0.0, "C": 0.0}
    w = {"S": ns / tot, "C": ncr / tot}
    for _ in range(n_slot):
        for k in acc:
            acc[k] += w[k]
        pick = max(acc, key=lambda k: acc[k])
        acc[pick] -= 1.0
        out.append(pick)
    return out


def _register_exp_op():
    import concourse.dve_ops as dvo
    from concourse.dve_spec import (
        AluOp,
        Bin,
        C0,
        C1,
        C2,
        One,
        Spec,
        Src0,
        lower,
        sq,
    )
    from concourse.dve_uop import DveOpSpec

    name = "EXP_CORRECT_ANT"
    if name in dvo._SUB_OPCODE_FOR_NAME:
        return next(op for op in dvo.OPS if op.name == name)

    def _ref(in0, in1, s0, s1, imm2):
        i = np.asarray(in0, np.float32).view(np.int32)
        z = ((i & 0x007FFFFF) | 0x3F800000).view(np.float32)
        d = z - np.float32(s1)
        g = (d * d * np.float32(imm2) + np.float32(1.0)).astype(np.float32)
        return (np.asarray(in0, np.float32) * g).astype(np.float32)

    body = Src0 * (
        sq(Bin(AluOp.BITWISE_OR, Bin(AluOp.BITWISE_AND, Src0, C0), One) - C1) * C2
        + One
    )
    spec = Spec(body=body, reference=_ref)
    row = dvo._CUSTOM_DVE_ROW_BASE + len(dvo.OPS)
    assert row < 0x20
    dvo._SUB_OPCODE_FOR_NAME[name] = row
    shas = {}
    for ver in ("v3",):
        uops = lower(spec, ver=ver)
        shas[ver] = DveOpSpec(name=name, opcode=row, uops=uops, rd1_en=False).sha(
            ver
        )
    op = dvo.DveOp(name, spec, subdim=False, uops_sha=shas)
    dvo.OPS.append(op)
    dvo.CUSTOM_DVE_SPECS[name] = spec
    return op


def build_attention_bass(T=T_FULL, heads=HPC, split=(N_S, N_C)):
    import concourse.bass as bass
    import concourse.tile as tile
    from concourse import bacc, mybir
    from concourse.masks import make_identity

    exp_op = _register_exp_op()

    f32 = mybir.dt.float32
    bf16 = mybir.dt.bfloat16
    i32 = mybir.dt.int32
    P = 128
    HALF = 512                  # columns per matmul (one PSUM bank of f32)
    TCB = min(1024, T)          # t-chunk (columns per shared weight load)
    NH = TCB // HALF            # matmuls per weight load
    SB = 128                    # s-block (rows per score matmul output)
    n_tc = T // TCB
    n_sb = T // SB
    n_tt = T // P               # t-tiles per head
    LAG = 3

    # exp stream constants (0.125 score scale folded in)
    SC_A = 0.125 * _LOG2E * 8388608.0
    BC_C = _B_C + 0.5  # +0.5 makes truncate-on-convert equal round-to-nearest

    assign = _make_assignment(n_sb, *split)

    nc = bacc.Bacc(
        "TRN2", target_bir_lowering=False, debug=False, num_devices=N_CORES
    )

    q_d = nc.dram_tensor("q", [heads, T, C], f32, kind="ExternalInput").ap()
    k_d = nc.dram_tensor("k", [heads, T, C], f32, kind="ExternalInput").ap()
    v_d = nc.dram_tensor("v", [heads, T, C], f32, kind="ExternalInput").ap()
    o_d = nc.dram_tensor("out", [heads, T, C], f32, kind="ExternalOutput").ap()

    with tile.TileContext(nc) as tc, ExitStack() as ctx:
        const_pool = ctx.enter_context(tc.tile_pool(name="const", bufs=1))
        stage_pool = ctx.enter_context(tc.tile_pool(name="stage", bufs=4))
        qkt_pool = ctx.enter_context(tc.tile_pool(name="qkt", bufs=4))
        vp_pool = ctx.enter_context(tc.tile_pool(name="vp", bufs=2))
        pt_pool = ctx.enter_context(tc.tile_pool(name="pt", bufs=5))
        it_pool = ctx.enter_context(tc.tile_pool(name="it", bufs=3))
        accT_pool = ctx.enter_context(tc.tile_pool(name="accT", bufs=3))
        out_pool = ctx.enter_context(tc.tile_pool(name="outsb", bufs=3))
        rec_pool = ctx.enter_context(tc.tile_pool(name="rec", bufs=3))
        psc = ctx.enter_context(tc.tile_pool(name="psc", bufs=2, space="PSUM"))
        psm = ctx.enter_context(tc.tile_pool(name="psm", bufs=2, space="PSUM"))

        # identities: bulk memset on the (idle) Vector engine; only the
        # cheap affine_select rides gpsimd, slotted between staging kicks.
        identb = const_pool.tile([P, P], bf16, tag="identb")
        ident = const_pool.tile([P, P], f32, tag="ident")
        nc.vector.memset(identb[:], 0.0)
        nc.vector.memset(ident[:], 0.0)

        def ident_select(t):
            nc.gpsimd.affine_select(
                out=t,
                in_=t,
                compare_op=mybir.AluOpType.not_equal,
                fill=1.0,
                base=0,
                pattern=[[-1, P]],
                channel_multiplier=1,
            )

        nq = n_tt // 4  # staging quarter
        q_sts, k_sts, vsbs = [], [], []
        for h in range(heads):
            q_st = stage_pool.tile([P, n_tt, C], bf16, tag="stage")
            k_st = stage_pool.tile([P, n_tt, C], bf16, tag="stage")
            q_sts.append(q_st); k_sts.append(k_st)
            # V' = [V | ones] per s-block, bf16 so PV weight loads can be
            # shared (standalone ldweights rejects 4-byte dtypes); only
            # the ones-column needs the memset (DMA fills the rest)
            v_sb = vp_pool.tile([P, n_sb, C + 1], bf16, tag="vp")
            vsbs.append(v_sb)
            nc.vector.memset(v_sb[:, :, C : C + 1], 1.0)

        def kick_qk(st, src, lo, hi):
            nc.gpsimd.dma_start(
                st[:, lo:hi, :],
                src.rearrange("(n p) c -> p n c", p=P)[:, lo:hi, :],
            )

        def kick_v(h):
            v_rr = v_d[h].rearrange("(n p) c -> p n c", p=P)
            nc.gpsimd.dma_start(vsbs[h][:, :, 0:C], v_rr[:])

        # gpsimd staging order, by first use (software-DGE transfers run
        # serially in kick order): K-q0 and Q-q0 first so chunk 0's first
        # score groups start ~13us; remaining K quarters land while their
        # transposes ride chunk-0 slots 0/2/4; V-0 halves before the
        # first PV (~3 slots in); head 1 last.
        kick_qk(k_sts[0], k_d[0], 0, nq)
        ident_select(identb[:])
        kick_qk(q_sts[0], q_d[0], 0, nq)
        kick_qk(k_sts[0], k_d[0], nq, 2 * nq)
        kick_qk(k_sts[0], k_d[0], 2 * nq, 3 * nq)
        kick_qk(k_sts[0], k_d[0], 3 * nq, 4 * nq)
        kick_v(0)
        kick_qk(q_sts[0], q_d[0], nq, 4 * nq)
        ident_select(ident[:])
        for h in range(1, heads):
            kick_qk(k_sts[h], k_d[h], 0, n_tt // 2)
            kick_qk(k_sts[h], k_d[h], n_tt // 2, n_tt)
            kick_v(h)
            kick_qk(q_sts[h], q_d[h], 0, n_tt // 2)
            kick_qk(q_sts[h], q_d[h], n_tt // 2, n_tt)

        kts, qts = [], []
        for h in range(heads):
            kt = qkt_pool.tile([C, T], bf16, tag="qkt")
            qt = qkt_pool.tile([C, T], bf16, tag="qkt")
            kts.append(kt); qts.append(qt)

        def pe_transposes(h, which, j0, n):
            """PE-transpose staged tiles j0..j0+n-1 into qt/kt [64, T]:
            n transposes into one psc-ring tile + one batched copy."""
            st = (q_sts if which == "q" else k_sts)[h]
            dst = (qts if which == "q" else kts)[h]
            tp = psc.tile([C, n, P], bf16, tag="sc")
            for jj in range(n):
                nc.tensor.transpose(tp[:, jj, :], st[:, j0 + jj, :], identb[:])
            if (j0 // n) % 2 == 0:
                nc.scalar.activation(
                    dst[:, j0 * P : (j0 + n) * P],
                    tp[:],
                    mybir.ActivationFunctionType.Copy,
                )
            else:
                nc.vector.tensor_copy(dst[:, j0 * P : (j0 + n) * P], tp[:])

        NB = min(8, n_tt)  # transpose batch size
        # head 0: K^T quarter 0 + Q^T chunk 0 — the minimum to start
        # chunk 0 — via PE transposes (NB-tile batches, psc ring).
        pe_transposes(0, "k", 0, NB)
        for j0 in range(0, TCB // P, NB):
            pe_transposes(0, "q", j0, NB)

        # Everything else rides the same PE path, NB-tile batches emitted
        # at fixed slots inside head-0's main loop (the psc-ring
        # insertion steals one lookahead step — a short stall each).
        # K quarters 1-3 go in chunk-0's first slots, chasing staging.
        late_work = []
        for j0 in range(NB, n_sb, NB):
            late_work.append((0, "k", j0))
        for j0 in range(TCB // P, n_tt, NB):
            late_work.append((0, "q", j0))
        for h in range(1, heads):
            for j0 in range(0, n_tt, NB):
                late_work.append((h, "k", j0))
        for h in range(1, heads):
            for j0 in range(0, n_tt, NB):
                late_work.append((h, "q", j0))
        late_slots = (12, 20, 28) if n_sb >= 32 else (1, 2, 3)

        def emit_late(item):
            pe_transposes(*item, NB)

        # ---- main loops (heads sequential; LAG s-blocks of score
        # lookahead hide the exp latency before the PV matmuls consume
        # each pt).  Each chunk's epilogue (per 512-col half) is DEFERRED
        # into the next chunk's slot stream: per half X in L,R: stage 0
        # copy acc->accT, stage 1 transposes + reciprocal, stage 2
        # normalize + out DMA.
        def emit_epilogue(h, i, q, acc, stage, state, final=False):
            if stage == 0:
                accT = accT_pool.tile([C + 1, HALF], f32, tag="accT")
                nc.scalar.activation(
                    accT[:], acc[:], mybir.ActivationFunctionType.Copy
                )
                state["accT"] = accT
            elif stage == 1:
                accT = state["accT"]
                td4 = psm.tile(
                    [P, HALF // P, C + 1], f32,
                    tag="accL" if q == 0 else "accR",
                )
                for b in range(HALF // P):
                    nc.tensor.transpose(
                        td4[:, b, :],
                        accT[:, b * P : (b + 1) * P],
                        ident[0 : C + 1, 0 : C + 1],
                    )
                rec = rec_pool.tile([P, HALF // P, 1], f32, tag="rec")
                nc.vector.reciprocal(rec[:], td4[:, :, C : C + 1])
                state["td4"] = td4
                state["rec"] = rec
            else:
                td4, rec = state["td4"], state["rec"]
                osb = out_pool.tile([P, HALF // P, C], f32, tag="outsb")
                for b in range(HALF // P):
                    if final and b % 2 == 0:
                        # exposed at the kernel tail: split the normalize
                        # across Scalar and Vector
                        nc.scalar.activation(
                            osb[:, b, :],
                            td4[:, b, 0:C],
                            mybir.ActivationFunctionType.Copy,
                            scale=rec[:, b, :],
                        )
                    else:
                        nc.vector.tensor_scalar_mul(
                            osb[:, b, :],
                            td4[:, b, 0:C],
                            rec[:, b, :],
                        )
                o_r = o_d[h].rearrange("(n p) c -> p n c", p=P)
                nb = HALF // P
                nc.sync.dma_start(
                    o_r[:, (i * NH + q) * nb : (i * NH + q + 1) * nb, :], osb[:]
                )

        pts = {}  # (h, i, m) -> pt tile; shared so head h+1's warm-up
        # score groups ride head h's final-chunk tail slots (which emit
        # no lookahead) and the PE never drains at the head transition.

        def emit_score(h, i, m):
            kt, qt = kts[h], qts[h]
            sc = psc.tile([P, NH, HALF], f32, tag="sc")
            w = kt[:, m * SB : (m + 1) * SB]
            nc.tensor.ldweights(w)
            for q in range(NH):
                mi = nc.tensor.matmul(
                    sc[:, q, :],
                    lhsT=w,
                    rhs=qt[:, i * TCB + q * HALF : i * TCB + (q + 1) * HALF],
                    start=True,
                    stop=True,
                )
                mi.ins.ldweights = False
            pt = pt_pool.tile([P, NH, HALF], bf16, tag="pt")
            if assign[m] == "S":
                nc.scalar.activation(
                    pt[:],
                    sc[:],
                    mybir.ActivationFunctionType.Exp,
                    scale=0.125,
                )
            else:  # corrected: affine+convert, then mantissa polish
                it = it_pool.tile([P, NH, HALF], i32, tag="it")
                nc.vector.tensor_scalar(
                    it[:],
                    sc[:],
                    SC_A,
                    BC_C,
                    op0=mybir.AluOpType.mult,
                    op1=mybir.AluOpType.add,
                )
                nc.vector._custom_dve(
                    exp_op,
                    out=pt[:],
                    in0=it[:].bitcast(f32),
                    s0=_MASK,
                    s1=_Z0,
                    imm2=_CC,
                )
            pts[(h, i, m)] = pt

        LAGe = min(LAG, n_sb)
        for m in range(LAGe):  # warm-up: first LAGe blocks of head 0
            emit_score(0, 0, m)
        for h in range(heads):
            v_sb = vsbs[h]
            pending = None  # (i, [accs]) of the prev chunk awaiting epilogue
            for i in range(n_tc):
                accs = [
                    psm.tile(
                        [C + 1, HALF], f32,
                        tag="accL" if q == 0 else "accR",
                        name=f"acc{q}",
                    )
                    for q in range(NH)
                ]
                epi_states = [dict() for _ in range(NH)]
                for m in range(n_sb):
                    # emit the score block LAGe ahead (maybe next chunk's)
                    t = m + LAGe
                    if t < n_sb:
                        emit_score(h, i, t)
                    elif i + 1 < n_tc:
                        emit_score(h, i + 1, t - n_sb)
                    elif h + 1 < heads:
                        # head h+1's warm-up rides these tail slots
                        emit_score(h + 1, 0, t - n_sb)
                    if pending is not None and m < 3 * NH:
                        emit_epilogue(
                            h, pending[0], m // 3, pending[1][m // 3],
                            m % 3, epi_states[m // 3],
                        )
                        if m == 3 * NH - 1:
                            pending = None
                    if h == 0 and late_work and (
                        m in late_slots or (i == 0 and m in (0, 2, 4))
                    ):
                        emit_late(late_work.pop(0))
                    pt = pts.pop((h, i, m))
                    wv = v_sb[:, m, :]
                    nc.tensor.ldweights(wv)
                    for q in range(NH):
                        mi = nc.tensor.matmul(
                            accs[q][:],
                            lhsT=wv,
                            rhs=pt[:, q, :],
                            start=(m == 0),
                            stop=(m == n_sb - 1),
                        )
                        mi.ins.ldweights = False
                pending = (i, accs)

            # flush the head's last chunk
            final = h == heads - 1
            epi_states = [dict() for _ in range(NH)]
            for q in range(NH):
                for st in range(3):
                    emit_epilogue(
                        h, pending[0], q, pending[1][q], st, epi_states[q],
                        final=final and q == NH - 1,
                    )

    nc.compile()
    return nc


_NC_CACHE = {}


def _get_nc(T, heads):
    key = (T, heads, N_S, N_C)
    if key not in _NC_CACHE:
        _NC_CACHE[key] = build_attention_bass(T, heads)
    return _NC_CACHE[key]


def _install_ntff_hook():
    """Register the axon NTFF profile hook that this image's antenv lacks.
    Only used when kernel(trace=True); never on the grading path."""
    import sys
    import types

    try:
        from antenv.axon_hooks import get_axon_ntff_profile_hook  # noqa: F401

        return
    except ImportError:
        pass
    import antenv
    from trn_agent_boot.trn_boot import _ntff_profile_via_ctypes

    holder = [_ntff_profile_via_ctypes("/opt/axon/libaxon_pjrt.so")]
    mod = types.ModuleType("antenv.axon_hooks")
    mod.get_axon_ntff_profile_hook = lambda: holder[0]
    mod.set_axon_ntff_profile_hook = lambda h: holder.__setitem__(0, h)
    sys.modules["antenv.axon_hooks"] = mod
    antenv.axon_hooks = mod

    import concourse.bass_utils as bu

    bu.upload_artifacts = lambda tmpdir: tmpdir  # no bucket in this sandbox


def kernel(query, key, value, trace=False):
    from concourse.bass_utils import run_bass_kernel_spmd

    if trace:
        _install_ntff_hook()

    Bq, Hq, T, Cq = query.shape
    nh = Bq * Hq
    heads = nh // N_CORES
    q = np.ascontiguousarray(query.reshape(nh, T, Cq).astype(np.float32))
    k = np.ascontiguousarray(key.reshape(nh, T, Cq).astype(np.float32))
    v = np.ascontiguousarray(value.reshape(nh, T, Cq).astype(np.float32))

    nc = _get_nc(T, heads)
    in_maps = [
        {
            "q": q[i * heads : (i + 1) * heads],
            "k": k[i * heads : (i + 1) * heads],
            "v": v[i * heads : (i + 1) * heads],
        }
        for i in range(N_CORES)
    ]
    res = run_bass_kernel_spmd(
        nc, in_maps, core_ids=list(range(N_CORES)), trace=trace
    )
    out = np.concatenate([res.results[i]["out"] for i in range(N_CORES)], axis=0)
    if trace:
        kernel.last_results = res
    return out.reshape(Bq, Hq, T, Cq)
